# revision 13
# baseline (speedup 1.0000x reference)
"""Trainium2 Bass kernel for nn_Attention (B=4, S=2048, D=1024, DK=256).

Computation (reference, per batch b):
    qp = q @ Wq.T            [S, DK]
    kp = q @ Wk.T            [S, DK]
    scores = qp @ kp.T / sqrt(DK)
    attn = softmax(scores, axis=-1)
    out = attn @ q           (v = q)
    y = out @ Wv.T           [S, D]

Sharding: 8 cores = 4 batches x 2 query-halves. Each core handles one batch's
full key/value sequence and one 1024-row query half. The host "rolls" the
sequence per core so that the core's query half occupies rows 0..1023; since
softmax is invariant to key permutation this changes nothing numerically.

All matmul operands are bf16 (same 1 cyc/row PE rate as fp32r, half the
DMA/SBUF traffic, and LDWEIGHTS gets Fast Weight Load, which fp32 weights
disable). Softmax denominator math stays fp32. rel err ~5e-3.

DMA strategy: a DMA_DIRECT2D issue costs ~650ns of engine time and one HWDGE
queue sustains only ~150GB/s, so inputs are split across BOTH hardware DGE
queues (SP + Activation) in PE-consumption order: the d-blocks of the
wk/wq weights (Act) and of qt chunk 0 (SP) stream in quarters so the first
projection starts at ~10us; later chunks/qn/wvt land as lo/hi halves, one
half per queue. Everything lives in one persistent pool (no SBUF reuse), so
no DMA ever carries a pool-release wait that would block its queue. Output
tiles alternate between the two queues.

Per-core dataflow, software-pipelined against the DMA stream:
    kpT[e, s_k] = wkT.T @ qT              (per 512-col chunk, acc over d)
    qpT[e, s_q] = wqT.T @ qT[:, :1024]
    per s_q chunk of 512:
      scoresT[s_k, s_q] = kpT.T @ qpT     (16 k-tiles x 2 e-acc)
      expT = exp(scoresT / 16)            (ScalarE, PSUM->SBUF bf16, fused scale)
      denom: DVE-accumulate expT over k -> ones-matmul partition sum ->
             PE-transpose 128-blocks -> reciprocal -> recip[s_q part, 1]
      unnormT[d, s_q] = qn.T @ expT       (8 d-tiles x 16 k-acc, 2 groups of 4)
      y[s_q, e_out] = unnormT.T @ wvT     (8 d-acc)
      y *= recip (per-partition) -> DMA out
    denom chunk 1 runs AFTER y chunk 0: its serial DVE add chain would
    otherwise head-of-line-block y0's scale ops in the DVE FIFO.

PSUM discipline (8 banks): tag "acc" bufs=4 (qp accumulators, then unnorm
groups), tag "sc" bufs=3 (kp accumulators, score tiles, y tiles), "pd" 1.
"""

import ml_dtypes
import numpy as np

import concourse.mybir as mybir
import concourse.tile as tile
from concourse import bacc
from concourse.bass_utils import run_bass_kernel_spmd
from concourse.masks import make_identity

B, S, D, DK = 4, 2048, 1024, 256
SQ = S // 2  # query rows per core
P = 128
CH = 512  # s_q chunk width
NC = S // 512  # 4 qt column chunks
N_CORES = 8

FR = mybir.dt.float32r
F32 = mybir.dt.float32
BF = mybir.dt.bfloat16
NP_BF = ml_dtypes.bfloat16

KT = S // P  # 16 key tiles
DT = D // P  # 8 d tiles
ET = DK // P  # 2 e tiles

_PROGRAM = None


def _build_program():
    nc = bacc.Bacc(None, target_bir_lowering=False, debug=False)

    # One wide row-block per logical input; host packs so every DMA reads a
    # contiguous [128, X] block.
    wkq_d = nc.dram_tensor("wkq", [P, DT * 4 * P], BF, kind="ExternalInput")
    qt_d = nc.dram_tensor("qt", [NC * P, DT * 512], BF, kind="ExternalInput")
    qn_d = nc.dram_tensor("qn", [NC * P, 4 * D], BF, kind="ExternalInput")
    wvt_d = nc.dram_tensor("wvt", [P, DT * 2 * 512], BF, kind="ExternalInput")
    y_d = nc.dram_tensor("y", [SQ, D], F32, kind="ExternalOutput")

    with tile.TileContext(nc) as tc:
        with (
            tc.tile_pool(name="pp", bufs=1) as pp,
            tc.tile_pool(name="ps", bufs=1, space="PSUM") as ps,
        ):
            # ---- constants + warmup ----
            ones_f = pp.tile([P, 1], F32, tag="ones_f")
            nc.vector.memset(ones_f[:], 1.0)
            ones = pp.tile([P, 1], FR, tag="ones")
            nc.vector.tensor_copy(ones[:], ones_f[:])
            ident = pp.tile([P, P], F32, tag="ident")
            make_identity(nc, ident[:])
            # Warm the ACT exp table-set (~2.7us first-call cost) early.
            warm_act = pp.tile([P, 1], F32, tag="warm_act")
            nc.scalar.activation(
                warm_act[:], ones_f[:], mybir.ActivationFunctionType.Exp
            )
            # Bridge boot-end (~8.5us) to first-data-landed (~9.7us).
            warm_f = pp.tile([P, 512], F32, tag="warm_f")
            nc.vector.memset(warm_f[:], 1.0)
            warm_r = pp.tile([P, 512], BF, tag="warm_r")
            nc.vector.tensor_copy(warm_r[:], warm_f[:])
            pwarm = ps.tile([P, 512], F32, tag="sc", bufs=3, name="pwarm")
            for _ in range(2):
                nc.tensor.matmul(
                    pwarm[:], warm_r[:, :P], warm_r[:], start=True, stop=True
                )

            # ---- persistent arrays ----
            wkq = pp.tile([P, DT * 4 * P], BF, tag="wkq", name="wkq")
            qtt = [
                pp.tile([P, DT * 512], BF, tag="qtt", bufs=NC, name=f"qtt{n}")
                for n in range(NC)
            ]
            qng = [
                pp.tile([P, 4 * D], BF, tag="qng", bufs=4, name=f"qng{j}")
                for j in range(4)
            ]
            wvn = pp.tile([P, DT * 2 * 512], BF, tag="wvt", name="wvt")
            kpt = {
                (e, n): pp.tile([P, 512], BF, tag="kpt", bufs=ET * NC, name=f"kpt{e}_{n}")
                for e in range(ET)
                for n in range(NC)
            }
            qpt = {
                (e, c): pp.tile([P, CH], BF, tag="qpt", bufs=ET * 2, name=f"qpt{e}_{c}")
                for e in range(ET)
                for c in range(2)
            }
            expt = {}  # (chunk, k) -> tile, allocated on the fly (tag-rotated)

            # ---- input DMA stream: two HWDGE queues (SP=sync, Act=scalar),
            # pieces ordered by PE consumption time ----
            # wkq + qt chunk 0 in quarters (one queue each, same cadence)
            Q4 = DT * 512 // 4
            for qq in range(4):
                nc.scalar.dma_start(
                    wkq[:, qq * Q4 : (qq + 1) * Q4], wkq_d[:, qq * Q4 : (qq + 1) * Q4]
                )
                nc.sync.dma_start(
                    qtt[0][:, qq * Q4 : (qq + 1) * Q4],
                    qt_d[0:P, qq * Q4 : (qq + 1) * Q4],
                )
            # qt chunks 1-3 as lo/hi halves, one half per queue
            H = DT * 512 // 2
            for n in range(1, NC):
                nc.sync.dma_start(
                    qtt[n][:, :H], qt_d[n * P : (n + 1) * P, :H]
                )
                nc.scalar.dma_start(
                    qtt[n][:, H:], qt_d[n * P : (n + 1) * P, H:]
                )
            # qn groups as halves
            HQ = 2 * D
            for j in range(4):
                nc.sync.dma_start(qng[j][:, :HQ], qn_d[j * P : (j + 1) * P, :HQ])
                nc.scalar.dma_start(qng[j][:, HQ:], qn_d[j * P : (j + 1) * P, HQ:])
            # wvt halves
            HV = DT * 512
            nc.sync.dma_start(wvn[:, :HV], wvt_d[:, :HV])
            nc.scalar.dma_start(wvn[:, HV:], wvt_d[:, HV:])

            def qn_sl(k, d):
                return qng[k // 4][:, (k % 4) * D + d * P : (k % 4) * D + (d + 1) * P]

            # ---- helpers ----
            def scores_block(c, ks):
                """scoresT + exp for key tiles ks of chunk c."""
                for k in ks:
                    sc = ps.tile([P, CH], F32, tag="sc", bufs=3, name=f"sc{c}_{k}")
                    for e in range(ET):
                        nc.tensor.matmul(
                            sc[:],
                            kpt[e, k // 4][:, (k % 4) * P : (k % 4 + 1) * P],
                            qpt[e, c][:],
                            start=(e == 0),
                            stop=(e == ET - 1),
                        )
                    ex = pp.tile([P, CH], BF, tag="expt", bufs=20, name=f"ex{c}_{k}")
                    nc.scalar.activation(
                        ex[:], sc[:], mybir.ActivationFunctionType.Exp, scale=1.0 / 16.0
                    )
                    expt[c, k] = ex

            def proj_chunk(n, with_qp):
                """kp (and qp if with_qp) for qt column chunk n, acc over d."""
                pks = {
                    e: ps.tile([P, 512], F32, tag="sc", bufs=3, name=f"pk{e}_{n}")
                    for e in range(ET)
                }
                pqs = (
                    {
                        e: ps.tile([P, 512], F32, tag="acc", bufs=4, name=f"pq{e}_{n}")
                        for e in range(ET)
                    }
                    if with_qp
                    else {}
                )
                for d in range(DT):
                    rhs = qtt[n][:, d * 512 : (d + 1) * 512]
                    for e in range(ET):
                        nc.tensor.matmul(
                            pks[e][:],
                            wkq[:, d * 512 + e * P : d * 512 + (e + 1) * P],
                            rhs,
                            start=(d == 0),
                            stop=(d == DT - 1),
                        )
                        if with_qp:
                            nc.tensor.matmul(
                                pqs[e][:],
                                wkq[:, d * 512 + 2 * P + e * P : d * 512 + 2 * P + (e + 1) * P],
                                rhs,
                                start=(d == 0),
                                stop=(d == DT - 1),
                            )
                for e in range(ET):
                    nc.vector.tensor_copy(kpt[e, n][:], pks[e][:])
                    if with_qp:
                        nc.vector.tensor_copy(qpt[e, n][:], pqs[e][:])

            def denom_block(c):
                dacc = pp.tile([P, CH], F32, tag="dacc", bufs=1, name=f"dacc{c}")
                nc.vector.tensor_copy(dacc[:], expt[c, 0][:])
                for k in range(1, KT):
                    nc.vector.tensor_tensor(
                        dacc[:], dacc[:], expt[c, k][:], op=mybir.AluOpType.add
                    )
                daccr = pp.tile([P, CH], FR, tag="daccr", bufs=1, name=f"daccr{c}")
                nc.vector.tensor_copy(daccr[:], dacc[:])
                pd = ps.tile([1, CH], F32, tag="pd", bufs=1, name=f"pd{c}")
                nc.tensor.matmul(pd[:], ones[:], daccr[:], start=True, stop=True)
                drow = pp.tile([1, CH], F32, tag="drow", bufs=2, name=f"drow{c}")
                nc.vector.tensor_copy(drow[:], pd[:])
                pt = ps.tile([P, CH // P], F32, tag="pd", bufs=1, name=f"pt{c}")
                for j in range(CH // P):
                    nc.tensor.transpose(
                        pt[:, j : j + 1], drow[:1, j * P : (j + 1) * P], ident[:1, :1]
                    )
                recip = pp.tile([P, CH // P], F32, tag="recip", bufs=2, name=f"recip{c}")
                nc.vector.reciprocal(recip[:], pt[:])
                return recip

            def unnorm_block(c):
                unsb = []
                for g in range(2):
                    accs = [
                        ps.tile([P, CH], F32, tag="acc", bufs=4, name=f"un{c}_{g}_{i}")
                        for i in range(4)
                    ]
                    for k in range(KT):
                        for i in range(4):
                            d = g * 4 + i
                            nc.tensor.matmul(
                                accs[i][:],
                                qn_sl(k, d),
                                expt[c, k][:],
                                start=(k == 0),
                                stop=(k == KT - 1),
                            )
                    for i in range(4):
                        us = pp.tile([P, CH], BF, tag="unsb", bufs=8, name=f"us{c}_{g}_{i}")
                        nc.vector.tensor_copy(us[:], accs[i][:])
                        unsb.append(us)
                return unsb

            def y_block(c, unsb, recip):
                cs = c * CH
                for m in range(CH // P):
                    for n in range(D // 512):
                        yb = ps.tile([P, 512], F32, tag="sc", bufs=3, name=f"yb{c}_{m}_{n}")
                        for d in range(DT):
                            nc.tensor.matmul(
                                yb[:],
                                unsb[d][:, m * P : (m + 1) * P],
                                wvn[:, (d * 2 + n) * 512 : (d * 2 + n + 1) * 512],
                                start=(d == 0),
                                stop=(d == DT - 1),
                            )
                        ys = pp.tile([P, 512], F32, tag="ysb", bufs=4, name=f"ys{c}_{m}_{n}")
                        nc.vector.tensor_scalar_mul(ys[:], yb[:], recip[:, m : m + 1])
                        dst = y_d[cs + m * P : cs + (m + 1) * P, n * 512 : (n + 1) * 512]
                        if (m * 2 + n) % 2 == 0:
                            nc.sync.dma_start(dst, ys[:])
                        else:
                            nc.scalar.dma_start(dst, ys[:])

            # ---- schedule (trace order == PE priority order) ----
            proj_chunk(0, with_qp=True)
            scores_block(0, range(0, 4))
            proj_chunk(1, with_qp=True)
            scores_block(0, range(4, 8))
            proj_chunk(2, with_qp=False)
            scores_block(0, range(8, 12))
            proj_chunk(3, with_qp=False)
            scores_block(0, range(12, 16))
            recip0 = denom_block(0)
            unsb0 = unnorm_block(0)
            # chunk-1 scores fill the PE while qn/wvt tails stream in
            scores_block(1, range(0, 16))
            y_block(0, unsb0, recip0)
            recip1 = denom_block(1)
            unsb1 = unnorm_block(1)
            y_block(1, unsb1, recip1)

    nc.compile()
    return nc


def build_in_maps(q, Wq, Wk, Wv):
    q = np.asarray(q, dtype=np.float32).astype(NP_BF)

    # wkq row p, d-block of 512: [wk_d (2x128) | wq_d (2x128)]
    wkt = np.asarray(Wk, dtype=np.float32).astype(NP_BF).T.reshape(DT, P, DK)
    wqt = np.asarray(Wq, dtype=np.float32).astype(NP_BF).T.reshape(DT, P, DK)
    wkq = np.ascontiguousarray(
        np.concatenate([wkt, wqt], axis=2).transpose(1, 0, 2).reshape(P, DT * 2 * DK)
    )

    # wvt row p: 16 blocks [(dd,n)] of 512 cols of Wv.T
    wvt = np.ascontiguousarray(
        np.asarray(Wv, dtype=np.float32)
        .astype(NP_BF)
        .T.reshape(DT, P, 2, 512)
        .transpose(1, 0, 2, 3)
        .reshape(P, DT * 2 * 512)
    )

    in_maps = []
    for core in range(N_CORES):
        b, h = divmod(core, 2)
        qb = q[b]
        rolled = np.concatenate([qb[h * SQ : (h + 1) * SQ], qb[(1 - h) * SQ : (2 - h) * SQ]])
        qT = rolled.T  # [D, S]
        # qt block n, row p: 8 d-blocks of 512 cols = qT[d*128+p, n*512:(n+1)*512]
        qt_packed = np.ascontiguousarray(
            qT.reshape(DT, P, NC, 512).transpose(2, 1, 0, 3).reshape(NC * P, DT * 512)
        )
        # qn group j, row p: 4 k-tiles (k=4j..4j+3) of D cols = rolled[k*128+p, :]
        qn_packed = np.ascontiguousarray(
            rolled.reshape(4, 4, P, D).transpose(0, 2, 1, 3).reshape(4 * P, 4 * D)
        )
        in_maps.append(
            {
                "qn": qn_packed,
                "qt": qt_packed,
                "wkq": wkq,
                "wvt": wvt,
            }
        )
    return in_maps


def kernel(q, Wq, Wk, Wv):
    global _PROGRAM
    if _PROGRAM is None:
        _PROGRAM = _build_program()
    nc = _PROGRAM
    in_maps = build_in_maps(q, Wq, Wk, Wv)
    res = run_bass_kernel_spmd(nc, in_maps, list(range(N_CORES)))

    out = np.empty((B, S, D), dtype=np.float32)
    for core in range(N_CORES):
        b, h = divmod(core, 2)
        out[b, h * SQ : (h + 1) * SQ, :] = res.results[core]["y"]
    return out


# revision 15
# speedup vs baseline: 1.0371x; 1.0371x over previous
"""Trainium2 Bass kernel for nn_Attention (B=4, S=2048, D=1024, DK=256).

Computation (reference, per batch b):
    qp = q @ Wq.T            [S, DK]
    kp = q @ Wk.T            [S, DK]
    scores = qp @ kp.T / sqrt(DK)
    attn = softmax(scores, axis=-1)
    out = attn @ q           (v = q)
    y = out @ Wv.T           [S, D]

Sharding: 8 cores = 4 batches x 2 query-halves. Each core handles one batch's
full key/value sequence and one 1024-row query half. The host "rolls" the
sequence per core so that the core's query half occupies rows 0..1023; since
softmax is invariant to key permutation this changes nothing numerically.

All matmul operands are bf16 (same 1 cyc/row PE rate as fp32r, half the
DMA/SBUF traffic, and LDWEIGHTS gets Fast Weight Load, which fp32 weights
disable). Softmax denominator math stays fp32. rel err ~5e-3.

DMA strategy: a DMA_DIRECT2D issue costs ~650ns of engine time and one HWDGE
queue sustains only ~150GB/s, so inputs are split across BOTH hardware DGE
queues (SP + Activation) in PE-consumption order: the d-blocks of the
wk/wq weights (Act) and of qt chunk 0 (SP) stream in quarters so the first
projection starts at ~10us; later chunks/qn/wvt land as lo/hi halves, one
half per queue. Everything lives in one persistent pool (no SBUF reuse), so
no DMA ever carries a pool-release wait that would block its queue. Output
tiles alternate between the two queues.

Per-core dataflow, software-pipelined against the DMA stream:
    kpT[e, s_k] = wkT.T @ qT              (per 512-col chunk, acc over d)
    qpT[e, s_q] = wqT.T @ qT[:, :1024]
    per s_q chunk of 512:
      scoresT[s_k, s_q] = kpT.T @ qpT     (16 k-tiles x 2 e-acc)
      expT = exp(scoresT / 16)            (ScalarE, PSUM->SBUF bf16, fused scale)
      denom: DVE-accumulate expT over k -> ones-matmul partition sum ->
             PE-transpose 128-blocks -> reciprocal -> recip[s_q part, 1]
      unnormT[d, s_q] = qn.T @ expT       (8 d-tiles x 16 k-acc, 2 groups of 4)
      y[s_q, e_out] = unnormT.T @ wvT     (8 d-acc)
      y *= recip (per-partition) -> DMA out
    denom chunk 1 runs AFTER y chunk 0: its serial DVE add chain would
    otherwise head-of-line-block y0's scale ops in the DVE FIFO.

PSUM discipline (8 banks): tag "acc" bufs=4 (qp accumulators, then unnorm
groups), tag "sc" bufs=3 (kp accumulators, score tiles, y tiles), "pd" 1.
"""

import ml_dtypes
import numpy as np

import concourse.mybir as mybir
import concourse.tile as tile
from concourse import bacc
from concourse.bass_utils import run_bass_kernel_spmd
from concourse.masks import make_identity

B, S, D, DK = 4, 2048, 1024, 256
SQ = S // 2  # query rows per core
P = 128
CH = 512  # s_q chunk width
NC = S // 512  # 4 qt column chunks
N_CORES = 8

FR = mybir.dt.float32r
F32 = mybir.dt.float32
BF = mybir.dt.bfloat16
NP_BF = ml_dtypes.bfloat16

KT = S // P  # 16 key tiles
DT = D // P  # 8 d tiles
ET = DK // P  # 2 e tiles

_PROGRAM = None


def _build_program():
    nc = bacc.Bacc(None, target_bir_lowering=False, debug=False)

    # One wide row-block per logical input; host packs so every DMA reads a
    # contiguous [128, X] block.
    wkq_d = nc.dram_tensor("wkq", [P, DT * 4 * P], BF, kind="ExternalInput")
    qt_d = nc.dram_tensor("qt", [NC * P, DT * 512], BF, kind="ExternalInput")
    qn_d = nc.dram_tensor("qn", [NC * P, 4 * D], BF, kind="ExternalInput")
    wvt_d = nc.dram_tensor("wvt", [P, DT * 2 * 512], BF, kind="ExternalInput")
    y_d = nc.dram_tensor("y", [SQ, D], F32, kind="ExternalOutput")

    with tile.TileContext(nc) as tc:
        with (
            tc.tile_pool(name="pp", bufs=1) as pp,
            tc.tile_pool(name="ps", bufs=1, space="PSUM") as ps,
        ):
            # ---- constants + warmup ----
            ones_f = pp.tile([P, 1], F32, tag="ones_f")
            nc.vector.memset(ones_f[:], 1.0)
            ones = pp.tile([P, 1], FR, tag="ones")
            nc.vector.tensor_copy(ones[:], ones_f[:])
            ident = pp.tile([P, P], F32, tag="ident")
            make_identity(nc, ident[:])
            # Warm the ACT exp table-set (~2.7us first-call cost) early.
            warm_act = pp.tile([P, 1], F32, tag="warm_act")
            nc.scalar.activation(
                warm_act[:], ones_f[:], mybir.ActivationFunctionType.Exp
            )
            # Bridge boot-end (~8.5us) to first-data-landed (~9.7us).
            warm_f = pp.tile([P, 512], F32, tag="warm_f")
            nc.vector.memset(warm_f[:], 1.0)
            warm_r = pp.tile([P, 512], BF, tag="warm_r")
            nc.vector.tensor_copy(warm_r[:], warm_f[:])
            pwarm = ps.tile([P, 512], F32, tag="sc", bufs=3, name="pwarm")
            for _ in range(2):
                nc.tensor.matmul(
                    pwarm[:], warm_r[:, :P], warm_r[:], start=True, stop=True
                )

            # ---- persistent arrays ----
            wkq = pp.tile([P, DT * 4 * P], BF, tag="wkq", name="wkq")
            qtt = [
                pp.tile([P, DT * 512], BF, tag="qtt", bufs=NC, name=f"qtt{n}")
                for n in range(NC)
            ]
            qng = [
                pp.tile([P, 4 * D], BF, tag="qng", bufs=4, name=f"qng{j}")
                for j in range(4)
            ]
            wvn = pp.tile([P, DT * 2 * 512], BF, tag="wvt", name="wvt")
            kpt = {
                (e, n): pp.tile([P, 512], BF, tag="kpt", bufs=ET * NC, name=f"kpt{e}_{n}")
                for e in range(ET)
                for n in range(NC)
            }
            qpt = {
                (e, c): pp.tile([P, CH], BF, tag="qpt", bufs=ET * 2, name=f"qpt{e}_{c}")
                for e in range(ET)
                for c in range(2)
            }
            expt = {}  # (chunk, k) -> tile, allocated on the fly (tag-rotated)

            # ---- input DMA stream: two HWDGE queues (SP=sync, Act=scalar) ----
            # The Act queue shares the Scalar engine FIFO with the exp
            # activations, and scores PSUM rotation depends on exps — so Act
            # gets only 6 early issues (done by ~11us, before the first exp),
            # plus a second group placed later in program order where the
            # Scalar engine is idle (during unnorm chunk 0). Everything else
            # rides Sync, ordered by PE need time with ~3us margin.
            Q4 = DT * 512 // 4
            for qq in range(4):
                nc.scalar.dma_start(
                    wkq[:, qq * Q4 : (qq + 1) * Q4], wkq_d[:, qq * Q4 : (qq + 1) * Q4]
                )
                nc.sync.dma_start(
                    qtt[0][:, qq * Q4 : (qq + 1) * Q4],
                    qt_d[0:P, qq * Q4 : (qq + 1) * Q4],
                )
            H = DT * 512 // 2
            HQ = 2 * D
            HV = DT * 512
            for n in (1, 2):
                nc.sync.dma_start(qtt[n][:, :H], qt_d[n * P : (n + 1) * P, :H])
                nc.scalar.dma_start(qtt[n][:, H:], qt_d[n * P : (n + 1) * P, H:])
            nc.sync.dma_start(qtt[3][:], qt_d[3 * P : 4 * P, :])
            nc.sync.dma_start(qng[0][:], qn_d[0:P, :])
            nc.sync.dma_start(qng[1][:, :HQ], qn_d[P : 2 * P, :HQ])
            nc.sync.dma_start(qng[2][:, :HQ], qn_d[2 * P : 3 * P, :HQ])
            nc.sync.dma_start(qng[3][:, :HQ], qn_d[3 * P : 4 * P, :HQ])
            nc.sync.dma_start(wvn[:, :HV], wvt_d[:, :HV])

            def late_input_dmas():
                # Act-queue group 2: dispatched from the Scalar FIFO after
                # chunk-0's exps, while unnorm 0 keeps the PE busy.
                nc.scalar.dma_start(qng[1][:, HQ:], qn_d[P : 2 * P, HQ:])
                nc.scalar.dma_start(qng[2][:, HQ:], qn_d[2 * P : 3 * P, HQ:])
                nc.scalar.dma_start(qng[3][:, HQ:], qn_d[3 * P : 4 * P, HQ:])
                nc.scalar.dma_start(wvn[:, HV:], wvt_d[:, HV:])

            def qn_sl(k, d):
                return qng[k // 4][:, (k % 4) * D + d * P : (k % 4) * D + (d + 1) * P]

            # ---- helpers ----
            def scores_block(c, ks):
                """scoresT + exp for key tiles ks of chunk c."""
                for k in ks:
                    sc = ps.tile([P, CH], F32, tag="sc", bufs=3, name=f"sc{c}_{k}")
                    for e in range(ET):
                        nc.tensor.matmul(
                            sc[:],
                            kpt[e, k // 4][:, (k % 4) * P : (k % 4 + 1) * P],
                            qpt[e, c][:],
                            start=(e == 0),
                            stop=(e == ET - 1),
                        )
                    ex = pp.tile([P, CH], BF, tag="expt", bufs=20, name=f"ex{c}_{k}")
                    nc.scalar.activation(
                        ex[:], sc[:], mybir.ActivationFunctionType.Exp, scale=1.0 / 16.0
                    )
                    expt[c, k] = ex

            def proj_chunk(n, with_qp):
                """kp (and qp if with_qp) for qt column chunk n, acc over d."""
                pks = {
                    e: ps.tile([P, 512], F32, tag="sc", bufs=3, name=f"pk{e}_{n}")
                    for e in range(ET)
                }
                pqs = (
                    {
                        e: ps.tile([P, 512], F32, tag="acc", bufs=4, name=f"pq{e}_{n}")
                        for e in range(ET)
                    }
                    if with_qp
                    else {}
                )
                for d in range(DT):
                    rhs = qtt[n][:, d * 512 : (d + 1) * 512]
                    for e in range(ET):
                        nc.tensor.matmul(
                            pks[e][:],
                            wkq[:, d * 512 + e * P : d * 512 + (e + 1) * P],
                            rhs,
                            start=(d == 0),
                            stop=(d == DT - 1),
                        )
                        if with_qp:
                            nc.tensor.matmul(
                                pqs[e][:],
                                wkq[:, d * 512 + 2 * P + e * P : d * 512 + 2 * P + (e + 1) * P],
                                rhs,
                                start=(d == 0),
                                stop=(d == DT - 1),
                            )
                for e in range(ET):
                    nc.vector.tensor_copy(kpt[e, n][:], pks[e][:])
                    if with_qp:
                        nc.vector.tensor_copy(qpt[e, n][:], pqs[e][:])

            def denom_block(c):
                dacc = pp.tile([P, CH], F32, tag="dacc", bufs=1, name=f"dacc{c}")
                nc.vector.tensor_copy(dacc[:], expt[c, 0][:])
                for k in range(1, KT):
                    nc.vector.tensor_tensor(
                        dacc[:], dacc[:], expt[c, k][:], op=mybir.AluOpType.add
                    )
                daccr = pp.tile([P, CH], FR, tag="daccr", bufs=1, name=f"daccr{c}")
                nc.vector.tensor_copy(daccr[:], dacc[:])
                pd = ps.tile([1, CH], F32, tag="pd", bufs=1, name=f"pd{c}")
                nc.tensor.matmul(pd[:], ones[:], daccr[:], start=True, stop=True)
                drow = pp.tile([1, CH], F32, tag="drow", bufs=2, name=f"drow{c}")
                nc.vector.tensor_copy(drow[:], pd[:])
                pt = ps.tile([P, CH // P], F32, tag="pd", bufs=1, name=f"pt{c}")
                for j in range(CH // P):
                    nc.tensor.transpose(
                        pt[:, j : j + 1], drow[:1, j * P : (j + 1) * P], ident[:1, :1]
                    )
                recip = pp.tile([P, CH // P], F32, tag="recip", bufs=2, name=f"recip{c}")
                nc.vector.reciprocal(recip[:], pt[:])
                return recip

            def unnorm_block(c):
                unsb = []
                for g in range(2):
                    accs = [
                        ps.tile([P, CH], F32, tag="acc", bufs=4, name=f"un{c}_{g}_{i}")
                        for i in range(4)
                    ]
                    for k in range(KT):
                        for i in range(4):
                            d = g * 4 + i
                            nc.tensor.matmul(
                                accs[i][:],
                                qn_sl(k, d),
                                expt[c, k][:],
                                start=(k == 0),
                                stop=(k == KT - 1),
                            )
                    for i in range(4):
                        us = pp.tile([P, CH], BF, tag="unsb", bufs=8, name=f"us{c}_{g}_{i}")
                        nc.vector.tensor_copy(us[:], accs[i][:])
                        unsb.append(us)
                return unsb

            def y_block(c, unsb, recip):
                cs = c * CH
                for m in range(CH // P):
                    for n in range(D // 512):
                        yb = ps.tile([P, 512], F32, tag="sc", bufs=3, name=f"yb{c}_{m}_{n}")
                        for d in range(DT):
                            nc.tensor.matmul(
                                yb[:],
                                unsb[d][:, m * P : (m + 1) * P],
                                wvn[:, (d * 2 + n) * 512 : (d * 2 + n + 1) * 512],
                                start=(d == 0),
                                stop=(d == DT - 1),
                            )
                        ys = pp.tile([P, 512], F32, tag="ysb", bufs=4, name=f"ys{c}_{m}_{n}")
                        nc.vector.tensor_scalar_mul(ys[:], yb[:], recip[:, m : m + 1])
                        dst = y_d[cs + m * P : cs + (m + 1) * P, n * 512 : (n + 1) * 512]
                        if (m * 2 + n) % 2 == 0:
                            nc.sync.dma_start(dst, ys[:])
                        else:
                            nc.scalar.dma_start(dst, ys[:])

            # ---- schedule (trace order == PE priority order) ----
            proj_chunk(0, with_qp=True)
            scores_block(0, range(0, 4))
            proj_chunk(1, with_qp=True)
            scores_block(0, range(4, 8))
            proj_chunk(2, with_qp=False)
            scores_block(0, range(8, 12))
            proj_chunk(3, with_qp=False)
            scores_block(0, range(12, 16))
            recip0 = denom_block(0)
            late_input_dmas()
            unsb0 = unnorm_block(0)
            # chunk-1 scores fill the PE while qn/wvt tails stream in
            scores_block(1, range(0, 16))
            y_block(0, unsb0, recip0)
            recip1 = denom_block(1)
            unsb1 = unnorm_block(1)
            y_block(1, unsb1, recip1)

    nc.compile()
    return nc


def build_in_maps(q, Wq, Wk, Wv):
    q = np.asarray(q, dtype=np.float32).astype(NP_BF)

    # wkq row p, d-block of 512: [wk_d (2x128) | wq_d (2x128)]
    wkt = np.asarray(Wk, dtype=np.float32).astype(NP_BF).T.reshape(DT, P, DK)
    wqt = np.asarray(Wq, dtype=np.float32).astype(NP_BF).T.reshape(DT, P, DK)
    wkq = np.ascontiguousarray(
        np.concatenate([wkt, wqt], axis=2).transpose(1, 0, 2).reshape(P, DT * 2 * DK)
    )

    # wvt row p: 16 blocks [(dd,n)] of 512 cols of Wv.T
    wvt = np.ascontiguousarray(
        np.asarray(Wv, dtype=np.float32)
        .astype(NP_BF)
        .T.reshape(DT, P, 2, 512)
        .transpose(1, 0, 2, 3)
        .reshape(P, DT * 2 * 512)
    )

    in_maps = []
    for core in range(N_CORES):
        b, h = divmod(core, 2)
        qb = q[b]
        rolled = np.concatenate([qb[h * SQ : (h + 1) * SQ], qb[(1 - h) * SQ : (2 - h) * SQ]])
        qT = rolled.T  # [D, S]
        # qt block n, row p: 8 d-blocks of 512 cols = qT[d*128+p, n*512:(n+1)*512]
        qt_packed = np.ascontiguousarray(
            qT.reshape(DT, P, NC, 512).transpose(2, 1, 0, 3).reshape(NC * P, DT * 512)
        )
        # qn group j, row p: 4 k-tiles (k=4j..4j+3) of D cols = rolled[k*128+p, :]
        qn_packed = np.ascontiguousarray(
            rolled.reshape(4, 4, P, D).transpose(0, 2, 1, 3).reshape(4 * P, 4 * D)
        )
        in_maps.append(
            {
                "qn": qn_packed,
                "qt": qt_packed,
                "wkq": wkq,
                "wvt": wvt,
            }
        )
    return in_maps


def kernel(q, Wq, Wk, Wv):
    global _PROGRAM
    if _PROGRAM is None:
        _PROGRAM = _build_program()
    nc = _PROGRAM
    in_maps = build_in_maps(q, Wq, Wk, Wv)
    res = run_bass_kernel_spmd(nc, in_maps, list(range(N_CORES)))

    out = np.empty((B, S, D), dtype=np.float32)
    for core in range(N_CORES):
        b, h = divmod(core, 2)
        out[b, h * SQ : (h + 1) * SQ, :] = res.results[core]["y"]
    return out
